# revision 1
# baseline (speedup 1.0000x reference)
"""Trainium2 Bass kernel for nn_GroupAttentionLayer (sparse block attention).

Strategy (8 NeuronCores, SPMD):
  Query sharding: core i handles batch b=i//2, query-pixel half h=i%2
  (2048 query pixels each). Attention, Conv_K accumulator and CBL_Q are
  computed per-batch with channel-major layouts so every reduction lands
  on the natural engine axis:

    scores^T[k,q] = Qc[:,k].T @ Xq[:,q]          (PE, contract channels)
    E = exp(scores/8)                             (ACT, fused 1/8 scale)
    D_bcast = blockmap.T @ E                      (PE; per-64-block sums,
                                                   pre-broadcast over partitions)
    A = E * recip(D_bcast)                        (DVE/POOL split)
    agg^T[c,q] += x_block[k,:].T @ A              (PE, contract keys, PSUM acc,
                                                   Conv_K folded in as first matmul)

  Two collectives: AllReduce of BN_Q batch stats ([128,2]) and AllGather
  of the per-core z1 shard (1 MB/rank). The epilogue (BN1 + spatial
  softmax + CBL_O) runs redundantly on every core from the gathered
  full tensor, so no further syncs are needed.

Host side: shards/transposes inputs with numpy, assembles the full
output from core 0's channel-major result.
"""

import numpy as np

B, H, W, C = 4, 64, 64, 128
RF = 8
EPS = 1e-3
ALPHA = 0.1
N_CORES = 8
HWPIX = H * W            # 4096 pixels per batch
QSH = HWPIX * B // N_CORES  # 2048 query pixels per core
PW = W + 2               # 66, padded row width
PADN = PW * (H + 2)      # 4356 padded columns
NKT = HWPIX // 128       # 32 key tiles per batch
NQT = QSH // 512         # 4 query tiles per core
NCH = (HWPIX * B) // 512  # 32 epilogue chunks
F32 = None               # set on first build (mybir.dt.float32)

# 1 of every DVE_EVERY normalize-multiplies runs on DVE; the rest on POOL
DVE_EVERY = 3

DEBUG = False  # adds intermediate-tensor outputs for bisection

_CACHE = {}


def _build_program():
    import concourse.bacc as bacc
    import concourse.tile as tile
    from concourse import mybir

    f32 = mybir.dt.float32
    f32r = mybir.dt.float32r
    AF = mybir.ActivationFunctionType
    OP = mybir.AluOpType
    AX = mybir.AxisListType

    nc = bacc.Bacc("TRN2", target_bir_lowering=False, debug=False,
                   enable_asserts=True, num_devices=N_CORES)

    # per-core inputs
    d_xb = nc.dram_tensor("xb", [HWPIX, C], f32, kind="ExternalInput").ap()
    d_xqT = nc.dram_tensor("xqT", [C, QSH], f32, kind="ExternalInput").ap()
    d_xpadT = nc.dram_tensor("xpadT", [C, PADN], f32, kind="ExternalInput").ap()
    # shared inputs
    d_wq9 = nc.dram_tensor("wq9", [9, C, C], f32, kind="ExternalInput").ap()
    d_wk = nc.dram_tensor("wk", [C, C], f32, kind="ExternalInput").ap()
    d_wo = nc.dram_tensor("wo", [C, C], f32, kind="ExternalInput").ap()
    d_vecs = nc.dram_tensor("vecs", [6, C], f32, kind="ExternalInput").ap()
    d_bm = nc.dram_tensor("bm", [C, C], f32, kind="ExternalInput").ap()
    # output: full channel-major result (identical on every core)
    d_outT = nc.dram_tensor("outT", [C, B * HWPIX], f32, kind="ExternalOutput").ap()
    if DEBUG:
        d_dbg_qc = nc.dram_tensor("dbg_qc", [C, HWPIX], f32,
                                  kind="ExternalOutput").ap()
        d_dbg_z1 = nc.dram_tensor("dbg_z1", [C, QSH], f32,
                                  kind="ExternalOutput").ap()
        d_dbg_zfull = nc.dram_tensor("dbg_zfull", [C, B * HWPIX], f32,
                                     kind="ExternalOutput").ap()

    with tile.TileContext(nc) as tc:
        with tc.tile_pool(name="const", bufs=1) as const, \
             tc.tile_pool(name="big", bufs=1) as big, \
             tc.tile_pool(name="work", bufs=6) as work, \
             tc.tile_pool(name="tmp2", bufs=2) as tmp2p, \
             tc.tile_pool(name="zbig", bufs=1) as zbig, \
             tc.tile_pool(name="small", bufs=2) as small, \
             tc.tile_pool(name="ps", bufs=3, space="PSUM") as ps, \
             tc.tile_pool(name="psA", bufs=2, space="PSUM") as psA, \
             tc.tile_pool(name="dram", bufs=1, space="DRAM") as dram:

            # ---------------- loads ----------------
            Xpad = big.tile([C, PADN], f32r)
            nc.sync.dma_start(Xpad[:], d_xpadT[:].bitcast(f32r))
            Xq = big.tile([C, QSH], f32r)
            nc.sync.dma_start(Xq[:], d_xqT[:].bitcast(f32r))
            Xnat = big.tile([128, NKT, C], f32r)
            nc.scalar.dma_start(
                Xnat[:], d_xb.rearrange("(t p) c -> p t c", p=128).bitcast(f32r))
            Wq_s = const.tile([C, 9, C], f32r)
            nc.scalar.dma_start(
                Wq_s[:], d_wq9.rearrange("t ci co -> ci t co").bitcast(f32r))
            Wk_s = const.tile([C, C], f32r)
            nc.sync.dma_start(Wk_s[:], d_wk[:].bitcast(f32r))
            Wo_s = const.tile([C, C], f32r)
            nc.sync.dma_start(Wo_s[:], d_wo[:].bitcast(f32r))
            V = const.tile([C, 6], f32)
            nc.scalar.dma_start(V[:], d_vecs.rearrange("v c -> c v"))
            Bb = const.tile([C, C], f32r)
            nc.sync.dma_start(Bb[:], d_bm[:].bitcast(f32r))
            eps_t = const.tile([C, 1], f32)
            nc.vector.memset(eps_t[:], EPS)

            Xpv = Xpad[:].rearrange("p (r c) -> p r c", r=H + 2)

            # ---------------- CBL_Q: conv3x3 + batch stats ----------------
            # Conv output is produced directly in BLOCK-MAJOR key order:
            # chunk t covers block-row n=t, column order (m, p, q) so that
            # Qc column n*512 + m*64 + p*8 + q is pixel (8n+p, 8m+q). Each
            # 128-column slice of Qc is then two complete 8x8 blocks,
            # matching the blockmap and the host-side permutation of xb.
            Zq = zbig.tile([C, 8, 512], f32, tag="zbig",
                           padded_shape=[C, 8, B * HWPIX // 8])  # shares Z slot
            qstats = small.tile([C, 8, 6], f32)
            for t in range(8):
                pq = psA.tile([C, 512], f32, tag="agg")
                for tap in range(9):
                    dh, dw = tap // 3 - 1, tap % 3 - 1
                    rhs = Xpv[:, t * 8 + 1 + dh: t * 8 + 9 + dh,
                              1 + dw: 65 + dw].rearrange(
                                  "c p (m q) -> c m p q", m=8)
                    nc.tensor.matmul(pq[:], Wq_s[:, tap, :], rhs,
                                     start=(tap == 0), stop=(tap == 8))
                nc.vector.bn_stats(qstats[:, t, :], pq[:])
                nc.scalar.copy(Zq[:, t, :], pq[:])

            qmv = small.tile([C, 2], f32)
            nc.vector.bn_aggr(qmv[:], qstats[:])
            # partial sums for the global (cross-core) stats:
            #   sums[:,0] = mean * 4096 ; sums[:,1] = (var + mean^2) * 4096
            sums = small.tile([C, 2], f32)
            nc.vector.tensor_scalar_mul(sums[:, 0:1], qmv[:, 0:1], float(HWPIX))
            m2 = small.tile([C, 1], f32)
            nc.vector.tensor_mul(m2[:], qmv[:, 0:1], qmv[:, 0:1])
            nc.vector.tensor_add(m2[:], m2[:], qmv[:, 1:2])
            nc.vector.tensor_scalar_mul(sums[:, 1:2], m2[:], float(HWPIX))

            st_in = dram.tile([C, 2], f32)
            st_out = dram.tile([C, 2], f32, addr_space="Shared")
            nc.sync.dma_start(st_in[:], sums[:])
            nc.gpsimd.collective_compute(
                "AllReduce", mybir.AluOpType.add,
                replica_groups=[list(range(N_CORES))],
                ins=[st_in.opt()], outs=[st_out.opt()])
            gst = small.tile([C, 2], f32)
            nc.sync.dma_start(gst[:], st_out[:])

            # global mean / var (each batch appears twice in the sum)
            TOT = float(HWPIX * N_CORES)
            mean_g = small.tile([C, 1], f32)
            nc.vector.tensor_scalar_mul(mean_g[:], gst[:, 0:1], 1.0 / TOT)
            negvar = small.tile([C, 1], f32)
            # (mean*mean) - E[z^2]  ->  -var
            ez2 = small.tile([C, 1], f32)
            nc.vector.tensor_scalar_mul(ez2[:], gst[:, 1:2], 1.0 / TOT)
            nc.vector.scalar_tensor_tensor(negvar[:], mean_g[:], mean_g[:],
                                           ez2[:], op0=OP.mult,
                                           op1=OP.subtract)
            std = small.tile([C, 1], f32)
            nc.scalar.activation(std[:], negvar[:], AF.Sqrt,
                                 scale=-1.0, bias=eps_t[:])
            rstd = small.tile([C, 1], f32)
            nc.vector.reciprocal(rstd[:], std[:])
            aq = small.tile([C, 1], f32)
            nc.vector.tensor_mul(aq[:], rstd[:], V[:, 0:1])
            bq = small.tile([C, 1], f32)
            nc.vector.tensor_scalar(bq[:], mean_g[:], aq[:], -1.0,
                                    op0=OP.mult, op1=OP.mult)
            nc.vector.tensor_add(bq[:], bq[:], V[:, 1:2])

            # q = leaky(aq*z + bq); Zq is already block-major
            Qc = big.tile([C, HWPIX], f32r)
            Qv = Qc[:].rearrange("p (t f) -> p t f", f=512)
            for t in range(8):
                tmp = tmp2p.tile([C, 512], f32, tag="tmp2")
                nc.scalar.activation(tmp[:], Zq[:, t, :], AF.Identity,
                                     scale=aq[:], bias=bq[:])
                nc.vector.scalar_tensor_tensor(Qv[:, t, :], tmp[:], ALPHA,
                                               tmp[:], op0=OP.mult,
                                               op1=OP.max)

            # ---------------- attention main loop ----------------
            z1s = big.tile([C, NQT, 512], f32)
            AGW = QSH + 8
            ag_in = dram.tile([C, AGW], f32)
            ag_out = dram.tile([N_CORES * C, AGW], f32, addr_space="Shared")
            qs1 = small.tile([C, NQT, 6], f32)
            for qt in range(NQT):
                pagg = psA.tile([C, 512], f32, tag="agg")
                # Conv_K accumulator folded into the attention PSUM group
                nc.tensor.matmul(pagg[:], Wk_s[:],
                                 Xq[:, qt * 512:(qt + 1) * 512],
                                 start=True, stop=False)
                for kt in range(NKT):
                    psS = ps.tile([C, 512], f32, tag="s")
                    nc.tensor.matmul(psS[:], Qc[:, kt * 128:(kt + 1) * 128],
                                     Xq[:, qt * 512:(qt + 1) * 512],
                                     start=True, stop=True)
                    E = work.tile([C, 512], f32r, tag="E")
                    nc.scalar.activation(E[:], psS[:], AF.Exp, scale=1.0 / RF)
                    psD = ps.tile([C, 512], f32, tag="d")
                    nc.tensor.matmul(psD[:], Bb[:], E[:],
                                     start=True, stop=True)
                    R = work.tile([C, 512], f32, tag="R")
                    nc.vector.reciprocal_approx_fast(R[:], psD[:])
                    A = work.tile([C, 512], f32r, tag="A")
                    if kt % DVE_EVERY == DVE_EVERY - 1:
                        nc.vector.tensor_mul(A[:], E[:], R[:])
                    else:
                        nc.gpsimd.tensor_mul(A[:], E[:], R[:])
                    nc.tensor.matmul(pagg[:], Xnat[:, kt, :], A[:],
                                     start=False, stop=(kt == NKT - 1))
                nc.scalar.copy(z1s[:, qt, :], pagg[:])
                nc.vector.bn_stats(qs1[:, qt, :], pagg[:])
                nc.sync.dma_start(ag_in[:, qt * 512:(qt + 1) * 512],
                                  z1s[:, qt, :])

            sh_mv = small.tile([C, 2], f32)
            nc.vector.bn_aggr(sh_mv[:], qs1[:])
            sh_sums = small.tile([C, 2], f32)
            nc.vector.tensor_scalar_mul(sh_sums[:, 0:1], sh_mv[:, 0:1],
                                        float(QSH))
            shm2 = small.tile([C, 1], f32)
            nc.vector.tensor_mul(shm2[:], sh_mv[:, 0:1], sh_mv[:, 0:1])
            nc.vector.tensor_add(shm2[:], shm2[:], sh_mv[:, 1:2])
            nc.vector.tensor_scalar_mul(sh_sums[:, 1:2], shm2[:], float(QSH))
            nc.sync.dma_start(ag_in[:, QSH:QSH + 2], sh_sums[:])

            if DEBUG:
                nc.sync.dma_start(d_dbg_qc[:], Qc[:])
                nc.sync.dma_start(d_dbg_z1[:],
                                  z1s[:].rearrange("c a b -> c (a b)"))

            nc.gpsimd.collective_compute(
                "AllGather", mybir.AluOpType.bypass,
                replica_groups=[list(range(N_CORES))],
                ins=[ag_in.opt()], outs=[ag_out.opt()])

            # ---------------- epilogue (redundant on all cores) ------------
            Z = zbig.tile([C, B * HWPIX], f32, tag="zbig")
            ag_v = ag_out[:].rearrange("(r p) f -> p r f", r=N_CORES)
            Zr16 = Z[:].rearrange("p (r h f) -> p r h f", r=N_CORES, h=2)
            for r in range(N_CORES):
                for hh in range(2):
                    eng = nc.sync if (2 * r + hh) % 2 == 0 else nc.scalar
                    eng.dma_start(Zr16[:, r, hh, :],
                                  ag_v[:, r, hh * (QSH // 2):(hh + 1) * (QSH // 2)])
            Zv = Z[:].rearrange("p (t f) -> p t f", f=512)
            st8 = small.tile([C, N_CORES, 2], f32)
            nc.sync.dma_start(st8[:], ag_v[:, :, QSH:QSH + 2])
            if DEBUG:
                nc.sync.dma_start(d_dbg_zfull[:], Z[:])

            TOT1 = float(B * HWPIX)
            gsum1 = small.tile([C, 2], f32)
            nc.vector.tensor_reduce(
                gsum1[:], st8[:].rearrange("c r j -> c j r"),
                axis=AX.X, op=OP.add)
            mean1 = small.tile([C, 1], f32)
            nc.vector.tensor_scalar_mul(mean1[:], gsum1[:, 0:1], 1.0 / TOT1)
            ez21 = small.tile([C, 1], f32)
            nc.vector.tensor_scalar_mul(ez21[:], gsum1[:, 1:2], 1.0 / TOT1)
            negv1 = small.tile([C, 1], f32)
            nc.vector.scalar_tensor_tensor(negv1[:], mean1[:], mean1[:],
                                           ez21[:], op0=OP.mult,
                                           op1=OP.subtract)
            std1 = small.tile([C, 1], f32)
            nc.scalar.activation(std1[:], negv1[:], AF.Sqrt,
                                 scale=-1.0, bias=eps_t[:])
            rstd1 = small.tile([C, 1], f32)
            nc.vector.reciprocal(rstd1[:], std1[:])
            a1 = small.tile([C, 1], f32)
            nc.vector.tensor_mul(a1[:], rstd1[:], V[:, 2:3])
            b1 = small.tile([C, 1], f32)
            nc.vector.tensor_scalar(b1[:], mean1[:], a1[:], -1.0,
                                    op0=OP.mult, op1=OP.mult)
            nc.vector.tensor_add(b1[:], b1[:], V[:, 3:4])

            # exp(BN1(z)) in place, with per-chunk sums from the ACT pass
            esums = small.tile([C, NCH], f32)
            for t in range(NCH):
                nc.scalar.activation(Zv[:, t, :], Zv[:, t, :], AF.Exp,
                                     scale=a1[:], bias=b1[:],
                                     accum_out=esums[:, t:t + 1])
            rb = small.tile([C, B], f32)
            for b in range(B):
                bsum = small.tile([C, 1], f32, tag="bsum")
                nc.vector.tensor_reduce(
                    bsum[:], esums[:, b * 8:(b + 1) * 8],
                    axis=AX.X, op=OP.add)
                nc.vector.reciprocal(rb[:, b:b + 1], bsum[:])
            # CBL_O: softmax-normalize each chunk into a small fp32r
            # staging tile, 1x1 conv + batch stats, overwrite Z with the
            # conv output (the exp values are no longer needed).
            stO = big.tile([C, NCH, 6], f32)
            for t in range(NCH):
                Yt = tmp2p.tile([C, 512], f32r, tag="Yt")
                nc.vector.tensor_scalar_mul(Yt[:], Zv[:, t, :],
                                            rb[:, t // 8:t // 8 + 1])
                pzo = ps.tile([C, 512], f32, tag="s")
                nc.tensor.matmul(pzo[:], Wo_s[:], Yt[:],
                                 start=True, stop=True)
                nc.vector.bn_stats(stO[:, t, :], pzo[:])
                nc.scalar.copy(Zv[:, t, :], pzo[:])
            mvO = small.tile([C, 2], f32)
            nc.vector.bn_aggr(mvO[:], stO[:])
            stdO = small.tile([C, 1], f32)
            nc.scalar.activation(stdO[:], mvO[:, 1:2], AF.Sqrt,
                                 scale=1.0, bias=eps_t[:])
            rstdO = small.tile([C, 1], f32)
            nc.vector.reciprocal(rstdO[:], stdO[:])
            aO = small.tile([C, 1], f32)
            nc.vector.tensor_mul(aO[:], rstdO[:], V[:, 4:5])
            bO = small.tile([C, 1], f32)
            nc.vector.tensor_scalar(bO[:], mvO[:, 0:1], aO[:], -1.0,
                                    op0=OP.mult, op1=OP.mult)
            nc.vector.tensor_add(bO[:], bO[:], V[:, 5:6])

            for t in range(NCH):
                tmp = tmp2p.tile([C, 512], f32, tag="tmp2")
                nc.scalar.activation(tmp[:], Zv[:, t, :], AF.Identity,
                                     scale=aO[:], bias=bO[:])
                nc.vector.scalar_tensor_tensor(Zv[:, t, :], tmp[:], ALPHA,
                                               tmp[:], op0=OP.mult,
                                               op1=OP.max)
                eng = nc.sync if t % 2 == 0 else nc.scalar
                eng.dma_start(d_outT[:, t * 512:(t + 1) * 512],
                              Zv[:, t, :])

    nc.compile()
    return nc


def _get_runner():
    if "runner" in _CACHE:
        return _CACHE["runner"]
    import jax
    import numpy as np
    from jax.sharding import Mesh, PartitionSpec
    from jax.experimental.shard_map import shard_map
    from concourse import mybir
    from concourse.bass2jax import (_bass_exec_p, install_neuronx_cc_hook,
                                    partition_id_tensor)

    nc = _build_program()
    install_neuronx_cc_hook()

    in_names, out_names, out_avals, zero_outs = [], [], [], []
    partition_name = nc.partition_id_tensor.name if nc.partition_id_tensor else None
    for alloc in nc.m.functions[0].allocations:
        if not isinstance(alloc, mybir.MemoryLocationSet):
            continue
        name = alloc.memorylocations[0].name
        if alloc.kind == "ExternalInput":
            if name != partition_name:
                in_names.append(name)
        elif alloc.kind == "ExternalOutput":
            shape = tuple(alloc.tensor_shape)
            dtype = mybir.dt.np(alloc.dtype)
            out_names.append(name)
            out_avals.append(jax.core.ShapedArray(shape, dtype))
            zero_outs.append(np.zeros(shape, dtype))
    n_params = len(in_names)
    n_outs = len(out_avals)
    all_in_names = list(in_names) + list(out_names)
    if partition_name is not None:
        all_in_names.append(partition_name)

    def _body(*args):
        operands = list(args)
        if partition_name is not None:
            operands.append(partition_id_tensor())
        outs = _bass_exec_p.bind(
            *operands,
            out_avals=tuple(out_avals),
            in_names=tuple(all_in_names),
            out_names=tuple(out_names),
            lowering_input_output_aliases=(),
            sim_require_finite=True,
            sim_require_nnan=True,
            nc=nc,
        )
        return tuple(outs)

    donate = tuple(range(n_params, n_params + n_outs))
    try:
        devices = jax.devices("axon")[:N_CORES]
    except RuntimeError:
        devices = jax.devices()[:N_CORES]
    mesh = Mesh(np.asarray(devices), ("core",))
    in_specs = (PartitionSpec("core"),) * (n_params + n_outs)
    out_specs = (PartitionSpec("core"),) * n_outs
    sharded = jax.jit(
        shard_map(_body, mesh=mesh, in_specs=in_specs, out_specs=out_specs,
                  check_rep=False),
        donate_argnums=donate, keep_unused=True)

    def run(in_maps):
        per_core = [[np.asarray(m[name]) for name in in_names] for m in in_maps]
        concat_in = [np.concatenate([per_core[c][i] for c in range(N_CORES)],
                                    axis=0) for i in range(n_params)]
        concat_zeros = [np.zeros((N_CORES * z.shape[0], *z.shape[1:]), z.dtype)
                        for z in zero_outs]
        out_arrs = jax.block_until_ready(sharded(*concat_in, *concat_zeros))
        return [
            {name: np.asarray(out_arrs[i]).reshape(N_CORES, *out_avals[i].shape)[c]
             for i, name in enumerate(out_names)}
            for c in range(N_CORES)
        ]

    _CACHE["runner"] = run
    return run


def _make_blockmap():
    bm = np.zeros((C, C), np.float32)
    idx = np.arange(C)
    bm[(idx[:, None] // 64) == (idx[None, :] // 64)] = 1.0
    return bm


def kernel(x, Wq, bq, gq, btq, Wk, bk, g1, bt1, Wo, bo, go, bto):
    """Full inputs -> full output. Conv biases cancel inside training-mode
    BN (the mean subtraction removes any per-channel constant), so bq/bk/bo
    never enter the device program."""
    x = np.asarray(x, np.float32)
    run = _get_runner()

    wq9 = np.ascontiguousarray(
        np.asarray(Wq, np.float32).reshape(9, C, C))
    wk = np.ascontiguousarray(np.asarray(Wk, np.float32).reshape(C, C))
    wo = np.ascontiguousarray(np.asarray(Wo, np.float32).reshape(C, C))
    vecs = np.ascontiguousarray(np.stack([
        np.asarray(v, np.float32) for v in (gq, btq, g1, bt1, go, bto)]))
    bm = _make_blockmap()

    # block-major key permutation: tile kt=(n,j) holds blocks (n,2j),(n,2j+1)
    # with partition index mb*64 + p*8 + q  (see QcB view in _build_program)
    perm = np.arange(HWPIX).reshape(8, 8, 8, 8).transpose(0, 2, 1, 3).reshape(-1)

    in_maps = []
    for core in range(N_CORES):
        b, h = core // 2, core % 2
        xb = np.ascontiguousarray(x[b].reshape(HWPIX, C))
        xbT = xb.T  # [C, HWPIX]
        xqT = np.ascontiguousarray(xbT[:, h * QSH:(h + 1) * QSH])
        xpadT = np.zeros((C, H + 2, W + 2), np.float32)
        xpadT[:, 1:H + 1, 1:W + 1] = xbT.reshape(C, H, W)
        in_maps.append({
            "xb": np.ascontiguousarray(xb[perm]),
            "xqT": xqT,
            "xpadT": np.ascontiguousarray(xpadT.reshape(C, PADN)),
            "wq9": wq9, "wk": wk, "wo": wo, "vecs": vecs, "bm": bm,
        })

    res = run(in_maps)
    outT = res[0]["outT"]  # [C, B*HWPIX], identical on every core
    return np.ascontiguousarray(outT.T).reshape(B, H, W, C)



# revision 3
# speedup vs baseline: 2.3715x; 2.3715x over previous
"""Trainium2 Bass kernel for nn_GroupAttentionLayer (block attention).

Strategy (8 NeuronCores, SPMD):
  Query sharding: core i handles batch b=i//2, query-pixel half h=i%2
  (2048 query pixels each). Channel-major layouts throughout:

    scores^T[k,q] = Qc[:,k].T @ Xq[:,q]          (PE, contract channels)
    E = exp(scores/8)                             (ACT, fused 1/8 scale)
    D_bcast = blockmap.T @ E                      (PE; per-64-block sums,
                                                   pre-broadcast over partitions)
    A = E / D                                     (DVE/POOL true divide, bf16 out)
    agg^T[c,q] += x_block[k,:].T @ A              (PE bf16, contract keys, PSUM acc,
                                                   Conv_K folded in as first matmul)

  The main loop is explicitly software-pipelined (stage offsets S/E/D/V/G)
  so each engine's in-order stream never head-of-line blocks on the
  producer chain; PSUM rings: scores 3 + denom 3 + agg 2 = 8 banks.

  All cross-core syncs are tiny AllGathers of batch-norm partial sums
  (AllGather [C,w] is ~2x cheaper than AllReduce in the collective cost
  model); the epilogue (BN1 + per-batch spatial softmax + CBL_O) runs
  SHARDED: the softmax division is folded into the 1x1 conv weights
  (Wo rows scaled by 1/S_b), so each core only ever touches its own
  2048 query pixels. The host stitches the 8 output shards.

Host side: shards/transposes inputs with numpy, converts the attention
value matrix to bf16, assembles the full output from all 8 shards.
"""

import numpy as np

B, H, W, C = 4, 64, 64, 128
RF = 8
EPS = 1e-3
ALPHA = 0.1
N_CORES = 8
HWPIX = H * W            # 4096 pixels per batch
QSH = HWPIX * B // N_CORES  # 2048 query pixels per core
PW = W + 2               # 66, padded row width
PADN = PW * (H + 2)      # 4356 padded columns
NKT = HWPIX // 128       # 32 key tiles per batch
NQT = QSH // 512         # 4 query tiles per core

N_WARM = 72              # PE warm-up matmuls issued under the sync-1 collective

DEBUG = False

_CACHE = {}


def _build_program():
    import concourse.bacc as bacc
    import concourse.tile as tile
    from concourse import mybir

    f32 = mybir.dt.float32
    f32r = mybir.dt.float32r
    bf16 = mybir.dt.bfloat16
    AF = mybir.ActivationFunctionType
    OP = mybir.AluOpType
    AX = mybir.AxisListType

    nc = bacc.Bacc("TRN2", target_bir_lowering=False, debug=False,
                   enable_asserts=True, num_devices=N_CORES)

    # per-core inputs
    d_xb = nc.dram_tensor("xb", [HWPIX, C], bf16, kind="ExternalInput").ap()
    d_xqT = nc.dram_tensor("xqT", [C, QSH], f32, kind="ExternalInput").ap()
    d_xpadT = nc.dram_tensor("xpadT", [C, PADN], f32, kind="ExternalInput").ap()
    d_selb = nc.dram_tensor("selb", [C, B], f32, kind="ExternalInput").ap()
    # shared inputs
    d_wq9 = nc.dram_tensor("wq9", [9, C, C], f32, kind="ExternalInput").ap()
    d_wk = nc.dram_tensor("wk", [C, C], f32, kind="ExternalInput").ap()
    d_wo = nc.dram_tensor("wo", [C, C], f32, kind="ExternalInput").ap()
    d_vecs = nc.dram_tensor("vecs", [6, C], f32, kind="ExternalInput").ap()
    d_bm = nc.dram_tensor("bm", [C, C], f32, kind="ExternalInput").ap()
    # output: this core's shard, channel-major
    d_outT = nc.dram_tensor("outT", [C, QSH], f32, kind="ExternalOutput").ap()
    if DEBUG:
        d_dbg_qc = nc.dram_tensor("dbg_qc", [C, HWPIX], f32,
                                  kind="ExternalOutput").ap()
        d_dbg_z1 = nc.dram_tensor("dbg_z1", [C, QSH], f32,
                                  kind="ExternalOutput").ap()

    with tile.TileContext(nc) as tc:
        with tc.tile_pool(name="const", bufs=1) as const, \
             tc.tile_pool(name="big", bufs=1) as big, \
             tc.tile_pool(name="workE", bufs=4) as workE, \
             tc.tile_pool(name="workA", bufs=3) as workA, \
             tc.tile_pool(name="tmp2", bufs=2) as tmp2p, \
             tc.tile_pool(name="small", bufs=2) as small, \
             tc.tile_pool(name="ps", bufs=3, space="PSUM") as ps, \
             tc.tile_pool(name="psA", bufs=2, space="PSUM") as psA, \
             tc.tile_pool(name="dram", bufs=1, space="DRAM") as dram:

            # ---------------- loads ----------------
            # scalar queue: conv weights first (chunk 0 needs them), then
            # the attention operands
            Wq_s = const.tile([C, 9, C], f32r)
            nc.scalar.dma_start(
                Wq_s[:], d_wq9.rearrange("t ci co -> ci t co").bitcast(f32r))
            Xq = big.tile([C, QSH], f32r)
            nc.scalar.dma_start(Xq[:], d_xqT[:].bitcast(f32r))
            Xnat = big.tile([128, NKT, C], bf16)
            nc.scalar.dma_start(
                Xnat[:], d_xb.rearrange("(t p) c -> p t c", p=128))
            Bb = const.tile([C, C], f32r)
            nc.scalar.dma_start(Bb[:], d_bm[:].bitcast(f32r))
            Wk_s = const.tile([C, C], f32r)
            nc.scalar.dma_start(Wk_s[:], d_wk[:].bitcast(f32r))
            Wo_s = const.tile([C, C], f32)
            nc.scalar.dma_start(Wo_s[:], d_wo[:])
            # sync queue: padded image in 3 row-band pieces so conv chunk 0
            # starts after ~1/3 of the transfer
            Xpad = big.tile([C, PADN], f32r)
            Xpad_v = Xpad[:].rearrange("c (r w) -> c r w", r=H + 2)
            d_xpad_v = d_xpadT.rearrange("c (r w) -> c r w", r=H + 2)
            for lo, hi in ((0, 22), (22, 44), (44, 66)):
                nc.sync.dma_start(Xpad_v[:, lo:hi, :],
                                  d_xpad_v[:, lo:hi, :].bitcast(f32r))
            V = const.tile([C, 6], f32)
            nc.sync.dma_start(V[:], d_vecs.rearrange("v c -> c v"))
            selb = const.tile([C, B], f32)
            nc.sync.dma_start(selb[:], d_selb[:])
            eps_t = const.tile([C, 1], f32)
            nc.vector.memset(eps_t[:], EPS)

            Xpv = Xpad[:].rearrange("p (r c) -> p r c", r=H + 2)

            # ---------------- CBL_Q: conv3x3 + batch stats ----------------
            # Conv output in BLOCK-MAJOR key order: chunk t covers block-row
            # n=t, column order (m, p, q) so Qc column n*512+m*64+p*8+q is
            # pixel (8n+p, 8m+q); each 128-column slice is two 8x8 blocks,
            # matching the blockmap and the host permutation of xb.
            Zq = big.tile([C, 8, 512], f32)
            qstats = small.tile([C, 8, 6], f32)
            for t in range(8):
                pq = ps.tile([C, 512], f32, tag="s")
                for tap in range(9):
                    dh, dw = tap // 3 - 1, tap % 3 - 1
                    rhs = Xpv[:, t * 8 + 1 + dh: t * 8 + 9 + dh,
                              1 + dw: 65 + dw].rearrange(
                                  "c p (m q) -> c m p q", m=8)
                    nc.tensor.matmul(pq[:], Wq_s[:, tap, :], rhs,
                                     start=(tap == 0), stop=(tap == 8))
                nc.vector.bn_stats(qstats[:, t, :], pq[:])
                nc.scalar.copy(Zq[:, t, :], pq[:])

            qmv = small.tile([C, 2], f32)
            nc.vector.bn_aggr(qmv[:], qstats[:])
            # partial sums for the global stats:
            #   sums[:,0] = mean*4096 ; sums[:,1] = (var+mean^2)*4096
            sums1 = small.tile([C, 2], f32)
            nc.vector.tensor_scalar_mul(sums1[:, 0:1], qmv[:, 0:1], float(HWPIX))
            m2 = small.tile([C, 1], f32)
            nc.vector.tensor_mul(m2[:], qmv[:, 0:1], qmv[:, 0:1])
            nc.vector.tensor_add(m2[:], m2[:], qmv[:, 1:2])
            nc.vector.tensor_scalar_mul(sums1[:, 1:2], m2[:], float(HWPIX))

            def allgather(tag, src, w):
                """AllGather a [C, w] stat tile; returns SBUF [C, 8, w]."""
                st_in = dram.tile([C, w], f32, tag=f"{tag}_in", name=f"{tag}_in")
                st_out = dram.tile([N_CORES * C, w], f32, addr_space="Shared",
                                   tag=f"{tag}_out", name=f"{tag}_out")
                nc.sync.dma_start(st_in[:], src)
                nc.gpsimd.collective_compute(
                    "AllGather", mybir.AluOpType.bypass,
                    replica_groups=[list(range(N_CORES))],
                    ins=[st_in.opt()], outs=[st_out.opt()])
                gst = small.tile([C, N_CORES, w], f32, tag=f"{tag}_g",
                                 name=f"{tag}_g")
                nc.sync.dma_start(
                    gst[:], st_out[:].rearrange("(r c) w -> c r w", r=N_CORES))
                return gst

            def bn_affine(tag, gst, tot, gcol, bcol):
                """Global mean/var from gathered partial sums -> (a, b) with
                a = gamma*rstd, b = beta - mean*a. rstd via exp(-0.5*ln(var+eps))
                keeps ACT on the {exp,ln,lrelu,copy} table (no table swaps)."""
                gsum = small.tile([C, 2], f32, tag=f"{tag}_gs", name=f"{tag}_gs")
                nc.vector.tensor_reduce(
                    gsum[:], gst[:].rearrange("c r j -> c j r"),
                    axis=AX.X, op=OP.add)
                mean = small.tile([C, 1], f32, tag=f"{tag}_mean", name=f"{tag}_mean")
                nc.vector.tensor_scalar_mul(mean[:], gsum[:, 0:1], 1.0 / tot)
                ez2 = small.tile([C, 1], f32, tag=f"{tag}_ez2", name=f"{tag}_ez2")
                nc.vector.tensor_scalar_mul(ez2[:], gsum[:, 1:2], 1.0 / tot)
                negvar = small.tile([C, 1], f32, tag=f"{tag}_nv", name=f"{tag}_nv")
                nc.vector.scalar_tensor_tensor(negvar[:], mean[:], mean[:],
                                               ez2[:], op0=OP.mult,
                                               op1=OP.subtract)
                lnv = small.tile([C, 1], f32, tag=f"{tag}_ln", name=f"{tag}_ln")
                nc.scalar.activation(lnv[:], negvar[:], AF.Ln,
                                     scale=-1.0, bias=eps_t[:])
                rstd = small.tile([C, 1], f32, tag=f"{tag}_rstd", name=f"{tag}_rstd")
                nc.scalar.activation(rstd[:], lnv[:], AF.Exp, scale=-0.5)
                a = small.tile([C, 1], f32, tag=f"{tag}_a", name=f"{tag}_a")
                nc.vector.tensor_mul(a[:], rstd[:], V[:, gcol:gcol + 1])
                b = small.tile([C, 1], f32, tag=f"{tag}_b", name=f"{tag}_b")
                nc.vector.tensor_scalar(b[:], mean[:], a[:], -1.0,
                                        op0=OP.mult, op1=OP.mult)
                nc.vector.tensor_add(b[:], b[:], V[:, bcol:bcol + 1])
                return a, b

            gst1 = allgather("s1", sums1[:], 2)

            # PE warmers: keep the tensor engine's p-state ramped through the
            # collective window (results discarded)
            for _ in range(N_WARM):
                pw = ps.tile([C, 512], f32, tag="d")
                nc.tensor.matmul(pw[:], Wk_s[:], Xq[:, 0:512],
                                 start=True, stop=True)

            aq, bq = bn_affine("s1", gst1, float(HWPIX * N_CORES), 0, 1)

            # q = leaky(aq*z + bq), fused in one ACT op per chunk
            Qc = big.tile([C, HWPIX], f32r)
            Qv = Qc[:].rearrange("p (t f) -> p t f", f=512)
            for t in range(8):
                nc.scalar.activation(Qv[:, t, :], Zq[:, t, :], AF.Lrelu,
                                     scale=aq[:], bias=bq[:], alpha=ALPHA)

            # ---------------- attention main loop ----------------
            # software pipeline: step s covers (kt=s//2, qt=2*pair+s%2);
            # stages S(s) scores, E(s-1) exp, D(s-2) block-sums, V(s-3)
            # divide, G(s-4) aggregate.
            z1s = big.tile([C, NQT, 512], f32)
            qs1 = small.tile([C, NQT, 6], f32)
            NSTEP = 2 * NKT

            for pair in range(2):
                paggs = {}

                def S_stage(s):
                    kt, j = s // 2, s % 2
                    qt = 2 * pair + j
                    psS = ps.tile([C, 512], f32, tag="s", name="psS")
                    nc.tensor.matmul(psS[:], Qc[:, kt * 128:(kt + 1) * 128],
                                     Xq[:, qt * 512:(qt + 1) * 512],
                                     start=True, stop=True)
                    return psS

                def E_stage(s, psS):
                    Et = workE.tile([C, 512], f32r, tag="E", name="Et")
                    nc.scalar.activation(Et[:], psS[:], AF.Exp, scale=1.0 / RF)
                    return Et

                def D_stage(s, Et):
                    psD = ps.tile([C, 512], f32, tag="d", name="psD")
                    nc.tensor.matmul(psD[:], Bb[:], Et[:],
                                     start=True, stop=True)
                    return psD

                def V_stage(s, Et, psD):
                    At = workA.tile([C, 512], bf16, tag="A", name="At")
                    eng = nc.vector if s % 2 == 0 else nc.gpsimd
                    eng.tensor_tensor(At[:], Et[:], psD[:], op=OP.divide)
                    return At

                def G_stage(s, At):
                    kt, j = s // 2, s % 2
                    nc.tensor.matmul(paggs[j][:], Xnat[:, kt, :], At[:],
                                     start=False, stop=(kt == NKT - 1))

                live = {}
                for s in range(NSTEP + 4):
                    if s < NSTEP:
                        psS = S_stage(s)
                        live[s] = [psS, None, None, None]
                    if s == 2:
                        # Conv_K init for both qt of this pair (delayed so the
                        # pair-0 PSUM release is off the PE critical path)
                        for j in range(2):
                            pagg = psA.tile([C, 512], f32, tag="agg",
                                            name="pagg")
                            qt = 2 * pair + j
                            nc.tensor.matmul(pagg[:], Wk_s[:],
                                             Xq[:, qt * 512:(qt + 1) * 512],
                                             start=True, stop=False)
                            paggs[j] = pagg
                    if 1 <= s < NSTEP + 1:
                        live[s - 1][1] = E_stage(s - 1, live[s - 1][0])
                    if 2 <= s < NSTEP + 2:
                        live[s - 2][2] = D_stage(s - 2, live[s - 2][1])
                    if 3 <= s < NSTEP + 3:
                        live[s - 3][3] = V_stage(s - 3, live[s - 3][1],
                                                 live[s - 3][2])
                    if 4 <= s < NSTEP + 4:
                        G_stage(s - 4, live[s - 4][3])
                        del live[s - 4]

                for j in range(2):
                    qt = 2 * pair + j
                    nc.vector.bn_stats(qs1[:, qt, :], paggs[j][:])
                    nc.scalar.copy(z1s[:, qt, :], paggs[j][:])

            # partial sums for BN1 (each query pixel counted once globally)
            sh_mv = small.tile([C, 2], f32)
            nc.vector.bn_aggr(sh_mv[:], qs1[:])
            sums2 = small.tile([C, 2], f32)
            nc.vector.tensor_scalar_mul(sums2[:, 0:1], sh_mv[:, 0:1],
                                        float(QSH))
            shm2 = small.tile([C, 1], f32)
            nc.vector.tensor_mul(shm2[:], sh_mv[:, 0:1], sh_mv[:, 0:1])
            nc.vector.tensor_add(shm2[:], shm2[:], sh_mv[:, 1:2])
            nc.vector.tensor_scalar_mul(sums2[:, 1:2], shm2[:], float(QSH))

            if DEBUG:
                nc.sync.dma_start(d_dbg_qc[:], Qc[:])
                nc.sync.dma_start(d_dbg_z1[:],
                                  z1s[:].rearrange("c a b -> c (a b)"))

            gst2 = allgather("s2", sums2[:], 2)
            a1, b1 = bn_affine("s2", gst2, float(B * HWPIX), 2, 3)

            # ---------------- sharded epilogue ----------------
            # e = exp(BN1(z1)) on this core's shard, with per-chunk sums
            ez = big.tile([C, NQT, 512], f32r)
            esum = small.tile([C, NQT], f32)
            for t in range(NQT):
                nc.scalar.activation(ez[:, t, :], z1s[:, t, :], AF.Exp,
                                     scale=a1[:], bias=b1[:],
                                     accum_out=esum[:, t:t + 1])
            epart = small.tile([C, 1], f32)
            nc.vector.tensor_reduce(epart[:], esum[:], axis=AX.X, op=OP.add)

            gst3 = allgather("s3", epart[:], 1)
            # per-batch spatial-softmax denominators: ranks (2b, 2b+1) -> batch b
            sb4 = small.tile([C, B], f32)
            nc.vector.tensor_reduce(
                sb4[:], gst3[:].rearrange("c (b h) w -> c b (h w)", b=B),
                axis=AX.X, op=OP.add)
            sbm = small.tile([C, B], f32)
            nc.vector.tensor_mul(sbm[:], sb4[:], selb[:])
            sb = small.tile([C, 1], f32)
            nc.vector.tensor_reduce(sb[:], sbm[:], axis=AX.X, op=OP.add)
            rS = small.tile([C, 1], f32)
            nc.vector.reciprocal(rS[:], sb[:])
            # fold the softmax division into the conv: scale Wo's input rows
            WoS = const.tile([C, C], f32r)
            nc.vector.tensor_scalar_mul(WoS[:], Wo_s[:], rS[:])

            # CBL_O conv + stats
            z2s = big.tile([C, NQT, 512], f32)
            stO = small.tile([C, NQT, 6], f32)
            for t in range(NQT):
                pz = ps.tile([C, 512], f32, tag="s", name="pz")
                nc.tensor.matmul(pz[:], WoS[:], ez[:, t, :],
                                 start=True, stop=True)
                nc.vector.bn_stats(stO[:, t, :], pz[:])
                nc.scalar.copy(z2s[:, t, :], pz[:])

            mvO = small.tile([C, 2], f32)
            nc.vector.bn_aggr(mvO[:], stO[:])
            sums4 = small.tile([C, 2], f32)
            nc.vector.tensor_scalar_mul(sums4[:, 0:1], mvO[:, 0:1], float(QSH))
            om2 = small.tile([C, 1], f32)
            nc.vector.tensor_mul(om2[:], mvO[:, 0:1], mvO[:, 0:1])
            nc.vector.tensor_add(om2[:], om2[:], mvO[:, 1:2])
            nc.vector.tensor_scalar_mul(sums4[:, 1:2], om2[:], float(QSH))

            gst4 = allgather("s4", sums4[:], 2)
            aO, bO = bn_affine("s4", gst4, float(B * HWPIX), 4, 5)

            for t in range(NQT):
                outc = tmp2p.tile([C, 512], f32, tag="outc", name="outc")
                nc.scalar.activation(outc[:], z2s[:, t, :], AF.Lrelu,
                                     scale=aO[:], bias=bO[:], alpha=ALPHA)
                eng = nc.sync if t % 2 == 0 else nc.scalar
                eng.dma_start(d_outT[:, t * 512:(t + 1) * 512], outc[:])

    nc.compile()
    return nc


def _get_runner():
    if "runner" in _CACHE:
        return _CACHE["runner"]
    import jax
    import numpy as np
    from jax.sharding import Mesh, PartitionSpec
    from jax.experimental.shard_map import shard_map
    from concourse import mybir
    from concourse.bass2jax import (_bass_exec_p, install_neuronx_cc_hook,
                                    partition_id_tensor)

    nc = _build_program()
    install_neuronx_cc_hook()

    in_names, out_names, out_avals, zero_outs = [], [], [], []
    partition_name = nc.partition_id_tensor.name if nc.partition_id_tensor else None
    for alloc in nc.m.functions[0].allocations:
        if not isinstance(alloc, mybir.MemoryLocationSet):
            continue
        name = alloc.memorylocations[0].name
        if alloc.kind == "ExternalInput":
            if name != partition_name:
                in_names.append(name)
        elif alloc.kind == "ExternalOutput":
            shape = tuple(alloc.tensor_shape)
            dtype = mybir.dt.np(alloc.dtype)
            out_names.append(name)
            out_avals.append(jax.core.ShapedArray(shape, dtype))
            zero_outs.append(np.zeros(shape, dtype))
    n_params = len(in_names)
    n_outs = len(out_avals)
    all_in_names = list(in_names) + list(out_names)
    if partition_name is not None:
        all_in_names.append(partition_name)

    def _body(*args):
        operands = list(args)
        if partition_name is not None:
            operands.append(partition_id_tensor())
        outs = _bass_exec_p.bind(
            *operands,
            out_avals=tuple(out_avals),
            in_names=tuple(all_in_names),
            out_names=tuple(out_names),
            lowering_input_output_aliases=(),
            sim_require_finite=True,
            sim_require_nnan=True,
            nc=nc,
        )
        return tuple(outs)

    donate = tuple(range(n_params, n_params + n_outs))
    try:
        devices = jax.devices("axon")[:N_CORES]
    except RuntimeError:
        devices = jax.devices()[:N_CORES]
    mesh = Mesh(np.asarray(devices), ("core",))
    in_specs = (PartitionSpec("core"),) * (n_params + n_outs)
    out_specs = (PartitionSpec("core"),) * n_outs
    sharded = jax.jit(
        shard_map(_body, mesh=mesh, in_specs=in_specs, out_specs=out_specs,
                  check_rep=False),
        donate_argnums=donate, keep_unused=True)

    def run(in_maps):
        per_core = [[np.asarray(m[name]) for name in in_names] for m in in_maps]
        concat_in = [np.concatenate([per_core[c][i] for c in range(N_CORES)],
                                    axis=0) for i in range(n_params)]
        concat_zeros = [np.zeros((N_CORES * z.shape[0], *z.shape[1:]), z.dtype)
                        for z in zero_outs]
        out_arrs = jax.block_until_ready(sharded(*concat_in, *concat_zeros))
        return [
            {name: np.asarray(out_arrs[i]).reshape(N_CORES, *out_avals[i].shape)[c]
             for i, name in enumerate(out_names)}
            for c in range(N_CORES)
        ]

    _CACHE["runner"] = run
    return run


def _make_blockmap():
    bm = np.zeros((C, C), np.float32)
    idx = np.arange(C)
    bm[(idx[:, None] // 64) == (idx[None, :] // 64)] = 1.0
    return bm


def kernel(x, Wq, bq, gq, btq, Wk, bk, g1, bt1, Wo, bo, go, bto):
    """Full inputs -> full output. Conv biases cancel inside training-mode
    BN (the mean subtraction removes any per-channel constant), so bq/bk/bo
    never enter the device program."""
    import ml_dtypes
    x = np.asarray(x, np.float32)
    run = _get_runner()

    wq9 = np.ascontiguousarray(
        np.asarray(Wq, np.float32).reshape(9, C, C))
    wk = np.ascontiguousarray(np.asarray(Wk, np.float32).reshape(C, C))
    wo = np.ascontiguousarray(np.asarray(Wo, np.float32).reshape(C, C))
    vecs = np.ascontiguousarray(np.stack([
        np.asarray(v, np.float32) for v in (gq, btq, g1, bt1, go, bto)]))
    bm = _make_blockmap()

    # block-major key permutation: tile kt=(n,j) holds blocks (n,2j),(n,2j+1)
    # with partition index mb*64 + p*8 + q
    perm = np.arange(HWPIX).reshape(8, 8, 8, 8).transpose(0, 2, 1, 3).reshape(-1)

    in_maps = []
    for core in range(N_CORES):
        b, h = core // 2, core % 2
        xb = np.ascontiguousarray(x[b].reshape(HWPIX, C))
        xbT = xb.T  # [C, HWPIX]
        xqT = np.ascontiguousarray(xbT[:, h * QSH:(h + 1) * QSH])
        xpadT = np.zeros((C, H + 2, W + 2), np.float32)
        xpadT[:, 1:H + 1, 1:W + 1] = xbT.reshape(C, H, W)
        selb = np.zeros((C, B), np.float32)
        selb[:, b] = 1.0
        in_maps.append({
            "xb": np.ascontiguousarray(xb[perm]).astype(ml_dtypes.bfloat16),
            "xqT": xqT,
            "xpadT": np.ascontiguousarray(xpadT.reshape(C, PADN)),
            "selb": selb,
            "wq9": wq9, "wk": wk, "wo": wo, "vecs": vecs, "bm": bm,
        })

    res = run(in_maps)
    out = np.empty((B, HWPIX, C), np.float32)
    for core in range(N_CORES):
        b, h = core // 2, core % 2
        out[b, h * QSH:(h + 1) * QSH, :] = res[core]["outT"].T
    return out.reshape(B, H, W, C)


# revision 4
# speedup vs baseline: 2.3843x; 1.0054x over previous
"""Trainium2 Bass kernel for nn_GroupAttentionLayer (block attention).

Strategy (8 NeuronCores, SPMD):
  Query sharding: core i handles batch b=i//2, query-pixel half h=i%2
  (2048 query pixels each). Channel-major layouts throughout:

    scores^T[k,q] = Qc[:,k].T @ Xq[:,q]          (PE, contract channels)
    E = exp(scores/8)                             (ACT, fused 1/8 scale)
    D_bcast = blockmap.T @ E                      (PE; per-64-block sums,
                                                   pre-broadcast over partitions)
    A = E / D                                     (DVE/POOL true divide, bf16 out)
    agg^T[c,q] += x_block[k,:].T @ A              (PE bf16, contract keys, PSUM acc,
                                                   Conv_K folded in as first matmul)

  The main loop is explicitly software-pipelined (stage offsets S/E/D/V/G)
  so each engine's in-order stream never head-of-line blocks on the
  producer chain; PSUM rings: scores 3 + denom 3 + agg 2 = 8 banks.

  All cross-core syncs are tiny AllGathers of batch-norm partial sums
  (AllGather [C,w] is ~2x cheaper than AllReduce in the collective cost
  model); the epilogue (BN1 + per-batch spatial softmax + CBL_O) runs
  SHARDED: the softmax division is folded into the 1x1 conv weights
  (Wo rows scaled by 1/S_b), so each core only ever touches its own
  2048 query pixels. The host stitches the 8 output shards.

Host side: shards/transposes inputs with numpy, converts the attention
value matrix to bf16, assembles the full output from all 8 shards.
"""

import numpy as np

B, H, W, C = 4, 64, 64, 128
RF = 8
EPS = 1e-3
ALPHA = 0.1
N_CORES = 8
HWPIX = H * W            # 4096 pixels per batch
QSH = HWPIX * B // N_CORES  # 2048 query pixels per core
PW = W + 2               # 66, padded row width
PADN = PW * (H + 2)      # 4356 padded columns
NKT = HWPIX // 128       # 32 key tiles per batch
NQT = QSH // 512         # 4 query tiles per core

N_WARM = 72              # PE warm-up matmuls issued under the sync-1 collective

DEBUG = False

_CACHE = {}


def _build_program():
    import concourse.bacc as bacc
    import concourse.tile as tile
    from concourse import mybir

    f32 = mybir.dt.float32
    f32r = mybir.dt.float32r
    bf16 = mybir.dt.bfloat16
    AF = mybir.ActivationFunctionType
    OP = mybir.AluOpType
    AX = mybir.AxisListType

    nc = bacc.Bacc("TRN2", target_bir_lowering=False, debug=False,
                   enable_asserts=True, num_devices=N_CORES)

    # per-core inputs
    d_xb = nc.dram_tensor("xb", [HWPIX, C], bf16, kind="ExternalInput").ap()
    d_xqT = nc.dram_tensor("xqT", [C, QSH], f32, kind="ExternalInput").ap()
    d_xpadT = nc.dram_tensor("xpadT", [C, PADN], f32, kind="ExternalInput").ap()
    d_selb = nc.dram_tensor("selb", [C, B], f32, kind="ExternalInput").ap()
    # shared inputs
    d_wq9 = nc.dram_tensor("wq9", [9, C, C], f32, kind="ExternalInput").ap()
    d_wk = nc.dram_tensor("wk", [C, C], f32, kind="ExternalInput").ap()
    d_wo = nc.dram_tensor("wo", [C, C], f32, kind="ExternalInput").ap()
    d_vecs = nc.dram_tensor("vecs", [6, C], f32, kind="ExternalInput").ap()
    d_bm = nc.dram_tensor("bm", [C, C], f32, kind="ExternalInput").ap()
    # output: this core's shard, channel-major
    d_outT = nc.dram_tensor("outT", [C, QSH], f32, kind="ExternalOutput").ap()
    if DEBUG:
        d_dbg_qc = nc.dram_tensor("dbg_qc", [C, HWPIX], f32,
                                  kind="ExternalOutput").ap()
        d_dbg_z1 = nc.dram_tensor("dbg_z1", [C, QSH], f32,
                                  kind="ExternalOutput").ap()

    with tile.TileContext(nc) as tc:
        with tc.tile_pool(name="const", bufs=1) as const, \
             tc.tile_pool(name="big", bufs=1) as big, \
             tc.tile_pool(name="workE", bufs=4) as workE, \
             tc.tile_pool(name="workA", bufs=3) as workA, \
             tc.tile_pool(name="tmp2", bufs=2) as tmp2p, \
             tc.tile_pool(name="small", bufs=2) as small, \
             tc.tile_pool(name="ps", bufs=3, space="PSUM") as ps, \
             tc.tile_pool(name="psA", bufs=2, space="PSUM") as psA, \
             tc.tile_pool(name="dram", bufs=1, space="DRAM") as dram:

            # ---------------- loads ----------------
            # scalar queue: conv weights first (chunk 0 needs them), then
            # the attention operands
            Wq_s = const.tile([C, 9, C], f32r)
            nc.scalar.dma_start(
                Wq_s[:], d_wq9.rearrange("t ci co -> ci t co").bitcast(f32r))
            Xq = big.tile([C, QSH], f32r)
            nc.scalar.dma_start(Xq[:], d_xqT[:].bitcast(f32r))
            Xnat = big.tile([128, NKT, C], bf16)
            nc.scalar.dma_start(
                Xnat[:], d_xb.rearrange("(t p) c -> p t c", p=128))
            Bb = const.tile([C, C], f32r)
            nc.scalar.dma_start(Bb[:], d_bm[:].bitcast(f32r))
            Wk_s = const.tile([C, C], f32r)
            nc.scalar.dma_start(Wk_s[:], d_wk[:].bitcast(f32r))
            Wo_s = const.tile([C, C], f32)
            nc.scalar.dma_start(Wo_s[:], d_wo[:])
            # sync queue: padded image in 3 row-band pieces so conv chunk 0
            # starts after ~1/3 of the transfer
            Xpad = big.tile([C, PADN], f32r)
            Xpad_v = Xpad[:].rearrange("c (r w) -> c r w", r=H + 2)
            d_xpad_v = d_xpadT.rearrange("c (r w) -> c r w", r=H + 2)
            for lo, hi in ((0, 22), (22, 44), (44, 66)):
                nc.sync.dma_start(Xpad_v[:, lo:hi, :],
                                  d_xpad_v[:, lo:hi, :].bitcast(f32r))
            V = const.tile([C, 6], f32)
            nc.sync.dma_start(V[:], d_vecs.rearrange("v c -> c v"))
            selb = const.tile([C, B], f32)
            nc.sync.dma_start(selb[:], d_selb[:])
            eps_t = const.tile([C, 1], f32)
            nc.vector.memset(eps_t[:], EPS)

            Xpv = Xpad[:].rearrange("p (r c) -> p r c", r=H + 2)

            # ---------------- CBL_Q: conv3x3 + batch stats ----------------
            # Conv output in BLOCK-MAJOR key order: chunk t covers block-row
            # n=t, column order (m, p, q) so Qc column n*512+m*64+p*8+q is
            # pixel (8n+p, 8m+q); each 128-column slice is two 8x8 blocks,
            # matching the blockmap and the host permutation of xb.
            Zq = big.tile([C, 8, 512], f32)
            qstats = small.tile([C, 8, 6], f32)
            for t in range(8):
                pq = ps.tile([C, 512], f32, tag="s")
                for tap in range(9):
                    dh, dw = tap // 3 - 1, tap % 3 - 1
                    rhs = Xpv[:, t * 8 + 1 + dh: t * 8 + 9 + dh,
                              1 + dw: 65 + dw].rearrange(
                                  "c p (m q) -> c m p q", m=8)
                    nc.tensor.matmul(pq[:], Wq_s[:, tap, :], rhs,
                                     start=(tap == 0), stop=(tap == 8))
                nc.vector.bn_stats(qstats[:, t, :], pq[:])
                nc.scalar.copy(Zq[:, t, :], pq[:])

            qmv = small.tile([C, 2], f32)
            nc.vector.bn_aggr(qmv[:], qstats[:])
            # partial sums for the global stats:
            #   sums[:,0] = mean*4096 ; sums[:,1] = (var+mean^2)*4096
            sums1 = small.tile([C, 2], f32)
            nc.vector.tensor_scalar_mul(sums1[:, 0:1], qmv[:, 0:1], float(HWPIX))
            m2 = small.tile([C, 1], f32)
            nc.vector.tensor_mul(m2[:], qmv[:, 0:1], qmv[:, 0:1])
            nc.vector.tensor_add(m2[:], m2[:], qmv[:, 1:2])
            nc.vector.tensor_scalar_mul(sums1[:, 1:2], m2[:], float(HWPIX))

            def allgather(tag, src, w):
                """AllGather a [C, w] stat tile; returns SBUF [C, 8, w]."""
                st_in = dram.tile([C, w], f32, tag=f"{tag}_in", name=f"{tag}_in")
                st_out = dram.tile([N_CORES * C, w], f32, addr_space="Shared",
                                   tag=f"{tag}_out", name=f"{tag}_out")
                nc.sync.dma_start(st_in[:], src)
                nc.gpsimd.collective_compute(
                    "AllGather", mybir.AluOpType.bypass,
                    replica_groups=[list(range(N_CORES))],
                    ins=[st_in.opt()], outs=[st_out.opt()])
                gst = small.tile([C, N_CORES, w], f32, tag=f"{tag}_g",
                                 name=f"{tag}_g")
                nc.sync.dma_start(
                    gst[:], st_out[:].rearrange("(r c) w -> c r w", r=N_CORES))
                return gst

            def bn_affine(tag, gst, tot, gcol, bcol):
                """Global mean/var from gathered partial sums -> (a, b) with
                a = gamma*rstd, b = beta - mean*a. rstd via exp(-0.5*ln(var+eps))
                keeps ACT on the {exp,ln,lrelu,copy} table (no table swaps)."""
                gsum = small.tile([C, 2], f32, tag=f"{tag}_gs", name=f"{tag}_gs")
                nc.vector.tensor_reduce(
                    gsum[:], gst[:].rearrange("c r j -> c j r"),
                    axis=AX.X, op=OP.add)
                mean = small.tile([C, 1], f32, tag=f"{tag}_mean", name=f"{tag}_mean")
                nc.vector.tensor_scalar_mul(mean[:], gsum[:, 0:1], 1.0 / tot)
                ez2 = small.tile([C, 1], f32, tag=f"{tag}_ez2", name=f"{tag}_ez2")
                nc.vector.tensor_scalar_mul(ez2[:], gsum[:, 1:2], 1.0 / tot)
                negvar = small.tile([C, 1], f32, tag=f"{tag}_nv", name=f"{tag}_nv")
                nc.vector.scalar_tensor_tensor(negvar[:], mean[:], mean[:],
                                               ez2[:], op0=OP.mult,
                                               op1=OP.subtract)
                lnv = small.tile([C, 1], f32, tag=f"{tag}_ln", name=f"{tag}_ln")
                nc.scalar.activation(lnv[:], negvar[:], AF.Ln,
                                     scale=-1.0, bias=eps_t[:])
                rstd = small.tile([C, 1], f32, tag=f"{tag}_rstd", name=f"{tag}_rstd")
                nc.scalar.activation(rstd[:], lnv[:], AF.Exp, scale=-0.5)
                a = small.tile([C, 1], f32, tag=f"{tag}_a", name=f"{tag}_a")
                nc.vector.tensor_mul(a[:], rstd[:], V[:, gcol:gcol + 1])
                b = small.tile([C, 1], f32, tag=f"{tag}_b", name=f"{tag}_b")
                nc.vector.tensor_scalar(b[:], mean[:], a[:], -1.0,
                                        op0=OP.mult, op1=OP.mult)
                nc.vector.tensor_add(b[:], b[:], V[:, bcol:bcol + 1])
                return a, b

            gst1 = allgather("s1", sums1[:], 2)

            # PE warmers: keep the tensor engine's p-state ramped through the
            # collective window (results discarded)
            for _ in range(N_WARM):
                pw = ps.tile([C, 512], f32, tag="d")
                nc.tensor.matmul(pw[:], Wk_s[:], Xq[:, 0:512],
                                 start=True, stop=True)

            aq, bq = bn_affine("s1", gst1, float(HWPIX * N_CORES), 0, 1)

            # q = leaky(aq*z + bq), fused in one ACT op per chunk
            Qc = big.tile([C, HWPIX], f32r)
            Qv = Qc[:].rearrange("p (t f) -> p t f", f=512)
            for t in range(8):
                nc.scalar.activation(Qv[:, t, :], Zq[:, t, :], AF.Prelu,
                                     scale=aq[:], bias=bq[:], alpha=ALPHA)

            # ---------------- attention main loop ----------------
            # software pipeline: step s covers (kt=s//2, qt=2*pair+s%2);
            # stages S(s) scores, E(s-1) exp, D(s-2) block-sums, V(s-3)
            # divide, G(s-4) aggregate.
            z1s = big.tile([C, NQT, 512], f32)
            qs1 = small.tile([C, NQT, 6], f32)
            NSTEP = 2 * NKT

            for pair in range(2):
                paggs = {}

                def S_stage(s):
                    kt, j = s // 2, s % 2
                    qt = 2 * pair + j
                    psS = ps.tile([C, 512], f32, tag="s", name="psS")
                    nc.tensor.matmul(psS[:], Qc[:, kt * 128:(kt + 1) * 128],
                                     Xq[:, qt * 512:(qt + 1) * 512],
                                     start=True, stop=True)
                    return psS

                def E_stage(s, psS):
                    Et = workE.tile([C, 512], f32r, tag="E", name="Et")
                    nc.scalar.activation(Et[:], psS[:], AF.Exp, scale=1.0 / RF)
                    return Et

                def D_stage(s, Et):
                    psD = ps.tile([C, 512], f32, tag="d", name="psD")
                    nc.tensor.matmul(psD[:], Bb[:], Et[:],
                                     start=True, stop=True)
                    return psD

                def V_stage(s, Et, psD):
                    # GPSIMD cannot touch PSUM, so every divide runs on DVE
                    At = workA.tile([C, 512], bf16, tag="A", name="At")
                    nc.vector.tensor_tensor(At[:], Et[:], psD[:], op=OP.divide)
                    return At

                def G_stage(s, At):
                    kt, j = s // 2, s % 2
                    nc.tensor.matmul(paggs[j][:], Xnat[:, kt, :], At[:],
                                     start=False, stop=(kt == NKT - 1))

                live = {}
                for s in range(NSTEP + 4):
                    if s < NSTEP:
                        psS = S_stage(s)
                        live[s] = [psS, None, None, None]
                    if s == 2:
                        # Conv_K init for both qt of this pair (delayed so the
                        # pair-0 PSUM release is off the PE critical path)
                        for j in range(2):
                            pagg = psA.tile([C, 512], f32, tag="agg",
                                            name="pagg")
                            qt = 2 * pair + j
                            nc.tensor.matmul(pagg[:], Wk_s[:],
                                             Xq[:, qt * 512:(qt + 1) * 512],
                                             start=True, stop=False)
                            paggs[j] = pagg
                    if 1 <= s < NSTEP + 1:
                        live[s - 1][1] = E_stage(s - 1, live[s - 1][0])
                    if 2 <= s < NSTEP + 2:
                        live[s - 2][2] = D_stage(s - 2, live[s - 2][1])
                    if 3 <= s < NSTEP + 3:
                        live[s - 3][3] = V_stage(s - 3, live[s - 3][1],
                                                 live[s - 3][2])
                    if 4 <= s < NSTEP + 4:
                        G_stage(s - 4, live[s - 4][3])
                        del live[s - 4]

                for j in range(2):
                    qt = 2 * pair + j
                    nc.vector.bn_stats(qs1[:, qt, :], paggs[j][:])
                    nc.scalar.copy(z1s[:, qt, :], paggs[j][:])

            # partial sums for BN1 (each query pixel counted once globally)
            sh_mv = small.tile([C, 2], f32)
            nc.vector.bn_aggr(sh_mv[:], qs1[:])
            sums2 = small.tile([C, 2], f32)
            nc.vector.tensor_scalar_mul(sums2[:, 0:1], sh_mv[:, 0:1],
                                        float(QSH))
            shm2 = small.tile([C, 1], f32)
            nc.vector.tensor_mul(shm2[:], sh_mv[:, 0:1], sh_mv[:, 0:1])
            nc.vector.tensor_add(shm2[:], shm2[:], sh_mv[:, 1:2])
            nc.vector.tensor_scalar_mul(sums2[:, 1:2], shm2[:], float(QSH))

            if DEBUG:
                nc.sync.dma_start(d_dbg_qc[:], Qc[:])
                nc.sync.dma_start(d_dbg_z1[:],
                                  z1s[:].rearrange("c a b -> c (a b)"))

            gst2 = allgather("s2", sums2[:], 2)
            a1, b1 = bn_affine("s2", gst2, float(B * HWPIX), 2, 3)

            # ---------------- sharded epilogue ----------------
            # e = exp(BN1(z1)) on this core's shard, with per-chunk sums
            ez = big.tile([C, NQT, 512], f32r)
            esum = small.tile([C, NQT], f32)
            for t in range(NQT):
                nc.scalar.activation(ez[:, t, :], z1s[:, t, :], AF.Exp,
                                     scale=a1[:], bias=b1[:],
                                     accum_out=esum[:, t:t + 1])
            epart = small.tile([C, 1], f32)
            nc.vector.tensor_reduce(epart[:], esum[:], axis=AX.X, op=OP.add)

            gst3 = allgather("s3", epart[:], 1)
            # per-batch spatial-softmax denominators: ranks (2b, 2b+1) -> batch b
            sb4 = small.tile([C, B], f32)
            nc.vector.tensor_reduce(
                sb4[:], gst3[:].rearrange("c (b h) w -> c b (h w)", b=B),
                axis=AX.X, op=OP.add)
            sbm = small.tile([C, B], f32)
            nc.vector.tensor_mul(sbm[:], sb4[:], selb[:])
            sb = small.tile([C, 1], f32)
            nc.vector.tensor_reduce(sb[:], sbm[:], axis=AX.X, op=OP.add)
            rS = small.tile([C, 1], f32)
            nc.vector.reciprocal(rS[:], sb[:])
            # fold the softmax division into the conv: scale Wo's input rows
            WoS = const.tile([C, C], f32r)
            nc.vector.tensor_scalar_mul(WoS[:], Wo_s[:], rS[:])

            # CBL_O conv + stats
            z2s = big.tile([C, NQT, 512], f32)
            stO = small.tile([C, NQT, 6], f32)
            for t in range(NQT):
                pz = ps.tile([C, 512], f32, tag="s", name="pz")
                nc.tensor.matmul(pz[:], WoS[:], ez[:, t, :],
                                 start=True, stop=True)
                nc.vector.bn_stats(stO[:, t, :], pz[:])
                nc.scalar.copy(z2s[:, t, :], pz[:])

            mvO = small.tile([C, 2], f32)
            nc.vector.bn_aggr(mvO[:], stO[:])
            sums4 = small.tile([C, 2], f32)
            nc.vector.tensor_scalar_mul(sums4[:, 0:1], mvO[:, 0:1], float(QSH))
            om2 = small.tile([C, 1], f32)
            nc.vector.tensor_mul(om2[:], mvO[:, 0:1], mvO[:, 0:1])
            nc.vector.tensor_add(om2[:], om2[:], mvO[:, 1:2])
            nc.vector.tensor_scalar_mul(sums4[:, 1:2], om2[:], float(QSH))

            gst4 = allgather("s4", sums4[:], 2)
            aO, bO = bn_affine("s4", gst4, float(B * HWPIX), 4, 5)

            for t in range(NQT):
                outc = tmp2p.tile([C, 512], f32, tag="outc", name="outc")
                nc.scalar.activation(outc[:], z2s[:, t, :], AF.Prelu,
                                     scale=aO[:], bias=bO[:], alpha=ALPHA)
                eng = nc.sync if t % 2 == 0 else nc.scalar
                eng.dma_start(d_outT[:, t * 512:(t + 1) * 512], outc[:])

    nc.compile()
    return nc


def _get_runner():
    if "runner" in _CACHE:
        return _CACHE["runner"]
    import jax
    import numpy as np
    from jax.sharding import Mesh, PartitionSpec
    from jax.experimental.shard_map import shard_map
    from concourse import mybir
    from concourse.bass2jax import (_bass_exec_p, install_neuronx_cc_hook,
                                    partition_id_tensor)

    nc = _build_program()
    install_neuronx_cc_hook()

    in_names, out_names, out_avals, zero_outs = [], [], [], []
    partition_name = nc.partition_id_tensor.name if nc.partition_id_tensor else None
    for alloc in nc.m.functions[0].allocations:
        if not isinstance(alloc, mybir.MemoryLocationSet):
            continue
        name = alloc.memorylocations[0].name
        if alloc.kind == "ExternalInput":
            if name != partition_name:
                in_names.append(name)
        elif alloc.kind == "ExternalOutput":
            shape = tuple(alloc.tensor_shape)
            dtype = mybir.dt.np(alloc.dtype)
            out_names.append(name)
            out_avals.append(jax.core.ShapedArray(shape, dtype))
            zero_outs.append(np.zeros(shape, dtype))
    n_params = len(in_names)
    n_outs = len(out_avals)
    all_in_names = list(in_names) + list(out_names)
    if partition_name is not None:
        all_in_names.append(partition_name)

    def _body(*args):
        operands = list(args)
        if partition_name is not None:
            operands.append(partition_id_tensor())
        outs = _bass_exec_p.bind(
            *operands,
            out_avals=tuple(out_avals),
            in_names=tuple(all_in_names),
            out_names=tuple(out_names),
            lowering_input_output_aliases=(),
            sim_require_finite=True,
            sim_require_nnan=True,
            nc=nc,
        )
        return tuple(outs)

    donate = tuple(range(n_params, n_params + n_outs))
    try:
        devices = jax.devices("axon")[:N_CORES]
    except RuntimeError:
        devices = jax.devices()[:N_CORES]
    mesh = Mesh(np.asarray(devices), ("core",))
    in_specs = (PartitionSpec("core"),) * (n_params + n_outs)
    out_specs = (PartitionSpec("core"),) * n_outs
    sharded = jax.jit(
        shard_map(_body, mesh=mesh, in_specs=in_specs, out_specs=out_specs,
                  check_rep=False),
        donate_argnums=donate, keep_unused=True)

    def run(in_maps):
        per_core = [[np.asarray(m[name]) for name in in_names] for m in in_maps]
        concat_in = [np.concatenate([per_core[c][i] for c in range(N_CORES)],
                                    axis=0) for i in range(n_params)]
        concat_zeros = [np.zeros((N_CORES * z.shape[0], *z.shape[1:]), z.dtype)
                        for z in zero_outs]
        out_arrs = jax.block_until_ready(sharded(*concat_in, *concat_zeros))
        return [
            {name: np.asarray(out_arrs[i]).reshape(N_CORES, *out_avals[i].shape)[c]
             for i, name in enumerate(out_names)}
            for c in range(N_CORES)
        ]

    _CACHE["runner"] = run
    return run


def _make_blockmap():
    bm = np.zeros((C, C), np.float32)
    idx = np.arange(C)
    bm[(idx[:, None] // 64) == (idx[None, :] // 64)] = 1.0
    return bm


def kernel(x, Wq, bq, gq, btq, Wk, bk, g1, bt1, Wo, bo, go, bto):
    """Full inputs -> full output. Conv biases cancel inside training-mode
    BN (the mean subtraction removes any per-channel constant), so bq/bk/bo
    never enter the device program."""
    import ml_dtypes
    x = np.asarray(x, np.float32)
    run = _get_runner()

    wq9 = np.ascontiguousarray(
        np.asarray(Wq, np.float32).reshape(9, C, C))
    wk = np.ascontiguousarray(np.asarray(Wk, np.float32).reshape(C, C))
    wo = np.ascontiguousarray(np.asarray(Wo, np.float32).reshape(C, C))
    vecs = np.ascontiguousarray(np.stack([
        np.asarray(v, np.float32) for v in (gq, btq, g1, bt1, go, bto)]))
    bm = _make_blockmap()

    # block-major key permutation: tile kt=(n,j) holds blocks (n,2j),(n,2j+1)
    # with partition index mb*64 + p*8 + q
    perm = np.arange(HWPIX).reshape(8, 8, 8, 8).transpose(0, 2, 1, 3).reshape(-1)

    in_maps = []
    for core in range(N_CORES):
        b, h = core // 2, core % 2
        xb = np.ascontiguousarray(x[b].reshape(HWPIX, C))
        xbT = xb.T  # [C, HWPIX]
        xqT = np.ascontiguousarray(xbT[:, h * QSH:(h + 1) * QSH])
        xpadT = np.zeros((C, H + 2, W + 2), np.float32)
        xpadT[:, 1:H + 1, 1:W + 1] = xbT.reshape(C, H, W)
        selb = np.zeros((C, B), np.float32)
        selb[:, b] = 1.0
        in_maps.append({
            "xb": np.ascontiguousarray(xb[perm]).astype(ml_dtypes.bfloat16),
            "xqT": xqT,
            "xpadT": np.ascontiguousarray(xpadT.reshape(C, PADN)),
            "selb": selb,
            "wq9": wq9, "wk": wk, "wo": wo, "vecs": vecs, "bm": bm,
        })

    res = run(in_maps)
    out = np.empty((B, HWPIX, C), np.float32)
    for core in range(N_CORES):
        b, h = core // 2, core % 2
        out[b, h * QSH:(h + 1) * QSH, :] = res[core]["outT"].T
    return out.reshape(B, H, W, C)
